# revision 1
# baseline (speedup 1.0000x reference)
"""CDMF segment-reduce kernel for 8 Trainium2 NeuronCores.

Strategy
--------
Host (cheap, index-only + one big gather):
  * stable-sort rows by user id; cut the 100k rows into 8 shards at user
    boundaries ("expert-style sharding of user segments") so each core owns a
    disjoint user range -> no cross-core reduction needed at all.
  * pad every shard to NT*128 rows (mask=0 rows contribute exactly 0).
  * compact each row's valid (mask=1) seq-slices to the front and zero the
    dead tail; masked slices are never used by the math, so they are not
    transferred at all. Per-tile chunk counts CH[t] (shared across all
    cores, the program is SPMD) bound the compacted width.
  * pre-gather q = item_emb[items] per shard. R and q ship as fp8 e3m4
    with exact power-of-2 pre-scales (compensated in w / the den column),
    one-hots as fp8 0/1, the rest bf16.
  * transpose R into PE-friendly chunks RT[(c%2)*64+d, tile, j, row] so the
    feature contraction runs on the tensor engine.
  * build per-tile one-hot matrices mapping the 128 rows of a tile to
    user-slots of a PSUM "bank" (bank b = users first seen in tile b).
    Only SLOTS (max users/tile, rounded up to 32) slots are kept.

Device (one SPMD program on 8 cores, DMA-roofline bound):
  * PE: per tile, CH[t] accumulating matmuls lhsT=RT-chunk [K=128=(2 slices
    x 64 d), M=128 rows], rhs=w-pattern [128, W] -> Z [128 rows, W] in PSUM.
  * DVE: one tensor_scalar (max(Z,tau), sum-accum) -> acc; padded columns
    contribute exactly tau each, so den = DEN_SCALE*wt = acc*cnt2 + corr
    with host-precomputed per-row constants (no mask tensor at all).
  * PE one-hot matmuls accumulate per-user [sum wt*q | DEN_SCALE*sum wt]
    (N=129) into PSUM banks; ACT flushes each bank to SBUF (bf16).
  * transposed one-hot matmuls gather num[user]/den[user] back per row;
    fused scalar_tensor_tensor computes sum_e num*q; per-GO-group
    reciprocal+mul finalizes r and streams it out on the idle Pool queue.
  * phase C lags phase A by LAG tiles inside one program-order loop so its
    DMAs interleave with the R stream instead of queueing after it.
"""

import numpy as np
import ml_dtypes

import concourse.bass as bass
import concourse.tile as tile
from concourse import bacc, mybir
from concourse.bass_utils import run_bass_kernel_spmd

N_CORES = 8
TAU = 0.01
S = 50          # seq_len
D = 64          # n_features
E = 128         # emb_dim
NJ = S // 2     # max PE k-chunks per tile (2 s-slices of 64 features each)
GO = 16          # one-hot / mask / q tiles per DMA
F32 = mybir.dt.float32
BF16 = mybir.dt.bfloat16
FP8 = mybir.dt.float8e4
# R stream dtype: fp8 e3m4 with an exact 2x pre-scale (2 folded into R,
# 1/2 into w) -- 4 mantissa bits halve the Z error vs e4m3 at equal bytes.
R_DT = mybir.dt.float8e3
R_SCALE = 2.0
R_CLIP = 15.5 / R_SCALE
# q stream: same e3m4 trick with a 32x pre-scale (item_emb ~ N(0, 0.1^2)).
# The two q factors in r = (sum wt*q)(q)/den make r scale by 32^2; scaling
# the den column of X by 32^2 cancels it exactly (powers of two).
Q_DT = mybir.dt.float8e3
NP_Q = mybir.dt.np(Q_DT)
Q_SCALE = 32.0
Q_CLIP = 15.5 / Q_SCALE
DEN_SCALE = Q_SCALE * Q_SCALE

NP_BF16 = ml_dtypes.bfloat16
NP_FP8 = mybir.dt.np(FP8)
NP_R = mybir.dt.np(R_DT)


# R-tile DMA group plan: small groups at both ends to shorten the pipeline
# prologue (first compute waits on a small DMA) and epilogue (last tile's
# chain starts as early as possible).
def _group_plan(NT):
    front, tail = [1, 1, 2], [2, 2, 1, 1]
    if NT <= sum(front) + sum(tail):
        plan, acc = [], 0
        for g in front + tail:
            if acc >= NT:
                break
            plan.append(min(g, NT - acc))
            acc += plan[-1]
        return plan
    mid = NT - sum(front) - sum(tail)
    plan = front + [5] * (mid // 5)
    if mid % 5:
        plan.append(mid % 5)
    return plan + tail


# Data-dependent build parameters (set by _preprocess, read by
# build_program's defaults so `build_program(NT)` builds the same program
# that kernel() runs).
_BUILD_PARAMS = None


# ----------------------------------------------------------------------------
# host-side preprocessing
# ----------------------------------------------------------------------------

def _preprocess(users, items, R_ui, mask, w, item_emb):
    global _BUILD_PARAMS
    n = users.shape[0]
    perm = np.argsort(users, kind="stable")
    users_s = users[perm]

    # shard cuts at user boundaries
    cuts = [0]
    for c in range(1, N_CORES):
        t = round(c * n / N_CORES)
        while 0 < t < n and users_s[t] == users_s[t - 1]:
            t += 1
        cuts.append(min(t, n))
    cuts.append(n)
    sizes = [cuts[c + 1] - cuts[c] for c in range(N_CORES)]
    NT = max(1, int(np.ceil(max(sizes) / 128)))
    NPAD = NT * 128

    q_full = item_emb[items]  # [n, E]
    w_bf = np.asarray(w, NP_BF16)

    # ---- pass 1: per-core masks -> shared chunk counts + slot count ----
    mks = []
    cnts = []
    slots_needed = 1
    metas = []
    row_maps = []
    for c in range(N_CORES):
        lo, hi = cuts[c], cuts[c + 1]
        nc_rows = hi - lo
        p0 = perm[lo:hi]
        u0 = users_s[lo:hi]
        # reorder users within the shard by their max valid-count so tiles
        # hold rows of similar width -- shrinks the per-tile chunk padding.
        # Only run-contiguity per user matters for the bank structure.
        cr = mask[p0].sum(1).astype(np.int64)
        uniq0, inv0 = np.unique(u0, return_inverse=True)
        umax = np.zeros(len(uniq0), np.int64)
        np.maximum.at(umax, inv0, cr)
        order = np.lexsort((np.arange(len(u0)), u0, umax[inv0]))
        p = p0[order]

        mk = np.zeros((NPAD, S), np.float32)
        mk[:nc_rows] = mask[p]
        mks.append(mk)
        cnts.append(mk.sum(1).astype(np.int64))
        metas.append((p, nc_rows))

        u = np.empty(NPAD, np.int64)
        u[:nc_rows] = u0[order]
        u[nc_rows:] = u[nc_rows - 1] if nc_rows > 0 else 0
        uniq, first_idx, inv = np.unique(u, return_index=True,
                                         return_inverse=True)
        ft = first_idx // 128
        # rank of each user within its bank (ft is not monotonic in user
        # value after the reorder, so rank via a stable sort of ft)
        fo = np.argsort(ft, kind="stable")
        ft_s = ft[fo]
        slot_s = np.arange(len(uniq)) - np.searchsorted(ft_s, ft_s, "left")
        slot = np.empty(len(uniq), np.int64)
        slot[fo] = slot_s
        slots_needed = max(slots_needed, int(np.bincount(ft).max()))
        row_maps.append((ft[inv], slot[inv]))

    cnt_mat = np.stack(cnts)                        # [cores, NPAD]
    tile_max = cnt_mat.reshape(N_CORES, NT, 128).max(-1).max(0)  # [NT]
    CH = np.maximum(1, (tile_max + 1) // 2).astype(np.int64)     # chunks/tile
    OFF = np.concatenate([[0], np.cumsum(CH)])       # chunk offsets
    TOT = int(OFF[-1])
    Wt = 2 * CH                                      # Z columns per tile
    WOFF = np.concatenate([[0], np.cumsum(Wt)])
    WTOT = int(WOFF[-1])
    SLOTS = int(min(128, ((slots_needed + 31) // 32) * 32))
    assert slots_needed <= 128, "bank overflow"

    _BUILD_PARAMS = {
        "NT": NT,
        "CH": tuple(int(x) for x in CH),
        "SLOTS": SLOTS,
    }

    # ---- pass 2: per-core arrays ----
    in_maps = []
    for c in range(N_CORES):
        p, nc_rows = metas[c]
        mk = mks[c]
        cnt = cnts[c]

        # compact valid slices to the front of each row (stable order)
        Rp = np.zeros((NPAD, S, D), NP_R)
        Rp[:nc_rows] = np.clip(R_ui[p], -R_CLIP, R_CLIP) * R_SCALE
        vidx = np.argsort(mk <= 0, axis=1, kind="stable")  # valid first
        cmp = np.take_along_axis(Rp, vidx[:, :, None], axis=1)
        # zero the dead tail so padded Z columns are exactly 0 -> max()=tau
        cmp[np.arange(S)[None, :] >= cnt[:, None]] = 0

        RT = np.empty((128, TOT * 128), NP_R)
        for t in range(NT):
            ch = int(CH[t])
            blk = cmp[t * 128:(t + 1) * 128, 0:2 * ch, :]
            RT[:, OFF[t] * 128:OFF[t + 1] * 128] = (
                blk.reshape(128, ch, 2, D).transpose(2, 3, 1, 0)
                .reshape(128, ch * 128)
            )

        # the compacted mask (column c of tile t live iff c < cnt[row]) is
        # built on-device from cnt via an iota compare -- only cnt ships.
        cntw = np.ascontiguousarray(
            cnt.reshape(NT, 128).T, np.float32)      # [128, NT]
        iota50 = np.ascontiguousarray(
            np.broadcast_to(np.arange(S, dtype=np.float32), (128, S))
        ).astype(NP_BF16)
        # fast path: padded Z columns contribute exactly tau each, so
        # wt = cnt*(acc - (W-cnt)*tau); ship cnt and the correction,
        # both pre-scaled by DEN_SCALE for the X den column.
        cm = cntw.astype(np.float32)
        Wrow = (2 * CH).astype(np.float32)[None, :]
        cnt2 = np.ascontiguousarray(cm * DEN_SCALE)
        corr = np.ascontiguousarray(
            np.float32(TAU) * cm * (cm - Wrow) * DEN_SCALE, np.float32)

        qp = np.zeros((NPAD, E), NP_Q)
        qp[:nc_rows] = np.clip(q_full[p], -Q_CLIP, Q_CLIP) * Q_SCALE
        qw = np.ascontiguousarray(qp.reshape(NT, 128, E).transpose(1, 0, 2))

        # w-pattern for the PE feature contraction: [128=(c%2)*64+d, NJ, S]
        # (carries the 1/R_SCALE compensation -- exact, power of two)
        wpat = np.zeros((128, NJ, S), NP_BF16)
        wb = (w_bf.astype(np.float32) / R_SCALE).astype(NP_BF16)
        for sp in range(2):
            for j in range(NJ):
                wpat[sp * 64:(sp + 1) * 64, j, 2 * j + sp] = wb

        fr, sr = row_maps[c]
        ii = np.arange(NPAD)
        tt, kk = ii // 128, ii % 128
        own = fr == tt
        prev = fr == tt - 1
        assert np.all(own | prev), "user spans >2 tiles (unexpected padding)"

        seg = np.zeros((128, NT, 2 * SLOTS), NP_FP8)
        gat = np.zeros((SLOTS, NT, 256), NP_FP8)
        seg[kk[own], tt[own], SLOTS + sr[own]] = 1.0
        seg[kk[prev], tt[prev], sr[prev]] = 1.0
        gat[sr[own], tt[own], 128 + kk[own]] = 1.0
        gat[sr[prev], tt[prev], kk[prev]] = 1.0

        in_maps.append(
            {
                "RT": RT,
                "cntw": cntw,
                "cnt2": cnt2,
                "corr": corr,
                "iota50": iota50,
                "qw": qw,
                "wpat": wpat,
                "ohs_seg": seg,
                "ohs_gat": gat,
            }
        )
    return in_maps, metas, NT


# ----------------------------------------------------------------------------
# device program
# ----------------------------------------------------------------------------

def build_program(NT, alpha=1.0, beta=1.0, gamma=1.0, params=None):
    if params is None:
        params = _BUILD_PARAMS
    if params is None or params["NT"] != NT:
        params = {"NT": NT, "CH": (NJ,) * NT, "SLOTS": 128}
    CH = params["CH"]
    SLOTS = params["SLOTS"]
    OFF = [0]
    for ch in CH:
        OFF.append(OFF[-1] + ch)
    TOT = OFF[-1]
    WOFF = [2 * o for o in OFF]
    WTOT = 2 * TOT

    nc = bacc.Bacc(
        "TRN2", target_bir_lowering=False, debug=False, num_devices=N_CORES
    )

    RT = nc.dram_tensor("RT", [128, TOT * 128], R_DT, kind="ExternalInput")
    cntw = nc.dram_tensor("cntw", [128, NT], F32, kind="ExternalInput")
    cnt2 = nc.dram_tensor("cnt2", [128, NT], F32, kind="ExternalInput")
    corr = nc.dram_tensor("corr", [128, NT], F32, kind="ExternalInput")
    iota50 = nc.dram_tensor("iota50", [128, S], BF16, kind="ExternalInput")
    qw = nc.dram_tensor("qw", [128, NT, E], Q_DT, kind="ExternalInput")
    wpat = nc.dram_tensor("wpat", [128, NJ, S], BF16, kind="ExternalInput")
    ohs_seg = nc.dram_tensor("ohs_seg", [128, NT, 2 * SLOTS], FP8,
                             kind="ExternalInput")
    ohs_gat = nc.dram_tensor("ohs_gat", [SLOTS, NT, 256], FP8,
                             kind="ExternalInput")
    r_out = nc.dram_tensor("r_out", [128, NT], F32, kind="ExternalOutput")

    fast = (alpha == 1.0) and (beta == 1.0) and (gamma == 1.0)
    AF = mybir.ActivationFunctionType

    plan = _group_plan(NT)
    gstart = [0]
    for g in plan:
        gstart.append(gstart[-1] + g)
    MAXC = max(OFF[gstart[i + 1]] - OFF[gstart[i]] for i in range(len(plan)))

    with tile.TileContext(nc) as tc:
        with (
            tc.tile_pool(name="const", bufs=1) as constp,
            tc.tile_pool(name="rpool", bufs=3) as rpool,
            tc.tile_pool(name="zpool", bufs=4) as zpool,
            tc.tile_pool(name="mpool", bufs=2) as mpool,
            tc.tile_pool(name="small", bufs=8) as small,
            tc.tile_pool(name="xpool", bufs=6) as xpool,
            tc.tile_pool(name="ohpool", bufs=3) as ohpool,
            tc.tile_pool(name="ohgpool", bufs=4) as ohgpool,
            tc.tile_pool(name="banks", bufs=1) as bankp,
            tc.tile_pool(name="psum_z", bufs=2, space="PSUM") as pz,
            tc.tile_pool(name="psum_seg", bufs=3, space="PSUM") as pseg,
            tc.tile_pool(name="psum_gat", bufs=3, space="PSUM") as pgat,
        ):
            wpat_sb = constp.tile([128, NJ, S], BF16)
            nc.sync.dma_start(wpat_sb[:], wpat[:, :, :])
            iota_sb = constp.tile([128, S], BF16)
            nc.sync.dma_start(iota_sb[:], iota50[:, :])
            qw_sb = constp.tile([128, NT, E], Q_DT)
            cnt_sb = constp.tile([128, NT], F32)
            nc.sync.dma_start(cnt_sb[:], cntw[:, :])
            cnt2_sb = constp.tile([128, NT], F32)
            nc.sync.dma_start(cnt2_sb[:], cnt2[:, :])
            corr_sb = constp.tile([128, NT], F32)
            nc.sync.dma_start(corr_sb[:], corr[:, :])
            den_sb = constp.tile([128, NT], F32)
            wt_sb = constp.tile([128, NT], F32)
            rn_sb = constp.tile([128, NT], F32)
            r_sb = constp.tile([128, NT], F32)
            bank_sb = bankp.tile([128, NT, 129], BF16)

            r_groups = {}
            oh_groups = {}
            bank_ps = [None] * NT
            tile_group = []
            for gi_, g in enumerate(plan):
                tile_group += [gi_] * g

            def phase_a(t):
                g = tile_group[t]
                if t == gstart[g]:
                    t1 = gstart[g + 1]
                    c0, c1 = OFF[t], OFF[t1]
                    rg = rpool.tile([128, MAXC * 128], R_DT)
                    nc.sync.dma_start(
                        rg[:, 0:(c1 - c0) * 128], RT[:, c0 * 128:c1 * 128]
                    )
                    r_groups[g] = rg
                og, ogi = divmod(t, GO)
                if ogi == 0:
                    ogn = min(GO, NT - t)
                    osg = ohpool.tile([128, GO, 2 * SLOTS], FP8)
                    nc.sync.dma_start(
                        osg[:, 0:ogn, :], ohs_seg[:, t:t + ogn, :]
                    )
                    oh_groups[og] = osg
                    nc.sync.dma_start(
                        qw_sb[:, t:t + ogn, :], qw[:, t:t + ogn, :]
                    )

                W = 2 * CH[t]
                rg = r_groups[g]
                base = (OFF[t] - OFF[gstart[g]]) * 128
                zps = pz.tile([128, S], F32)
                for j in range(CH[t]):
                    nc.tensor.matmul(
                        zps[:, 0:W], rg[:, base + j * 128:base + (j + 1) * 128],
                        wpat_sb[:, j, 0:W],
                        start=(j == 0), stop=(j == CH[t] - 1),
                    )

                wt_col = wt_sb[:, t:t + 1]
                cnt_col = cnt_sb[:, t:t + 1]
                if fast:
                    # acc = sum_c max(z, tau); padded columns add exactly tau
                    # so den = DEN_SCALE*wt = acc*cnt2 + corr (see host)
                    acc_col = small.tile([128, 1], F32, tag="acc")
                    wp = zpool.tile([128, S], BF16)
                    nc.vector.tensor_scalar(
                        wp[:, 0:W], zps[:, 0:W], TAU, None,
                        op0=mybir.AluOpType.max, op1=mybir.AluOpType.add,
                        accum_out=acc_col[:],
                    )
                else:
                    mct = mpool.tile([128, S], BF16)
                    mcol = mct[:, 0:W]
                    nc.vector.tensor_scalar(
                        mcol, iota_sb[:, 0:W], cnt_col, 1.0,
                        op0=mybir.AluOpType.is_lt, op1=mybir.AluOpType.mult,
                    )
                    z = zpool.tile([128, S], F32, tag="zf32")
                    nc.vector.tensor_scalar_max(z[:, 0:W], zps[:, 0:W], TAU)
                    # z <- exp(alpha * ln z)   (z >= TAU > 0)
                    nc.scalar.activation(z[:, 0:W], z[:, 0:W], AF.Log)
                    nc.scalar.activation(z[:, 0:W], z[:, 0:W], AF.Exp,
                                         scale=float(alpha))
                    wp = zpool.tile([128, S], F32, tag="wpf32")
                    nc.vector.tensor_mul(wp[:, 0:W], z[:, 0:W], mcol)
                    a_col = small.tile([128, 1], F32)
                    nc.vector.tensor_reduce(
                        a_col[:], wp[:, 0:W], axis=mybir.AxisListType.X,
                        op=mybir.AluOpType.add,
                    )
                    # wt = (A^(1/alpha) * cnt^beta)^gamma
                    #    = exp(gamma*(ln(A)/alpha + beta*ln(cnt)))
                    la = small.tile([128, 1], F32)
                    nc.scalar.activation(la[:], a_col[:], AF.Log)
                    lc = small.tile([128, 1], F32)
                    nc.scalar.activation(lc[:], cnt_sb[:, t:t + 1], AF.Log)
                    nc.vector.scalar_tensor_tensor(
                        la[:], lc[:], float(alpha * beta), la[:],
                        op0=mybir.AluOpType.mult, op1=mybir.AluOpType.add,
                    )
                    nc.scalar.activation(
                        wt_col, la[:], AF.Exp, scale=float(gamma / alpha)
                    )

                # X_t = [wt*q | DEN_SCALE*wt]; dwt_col = DEN_SCALE*wt (f32)
                xt = xpool.tile([128, 129], BF16)
                dwt_col = wt_sb[:, t:t + 1]
                if fast:
                    nc.vector.scalar_tensor_tensor(
                        dwt_col, acc_col[:], cnt2_sb[:, t:t + 1],
                        corr_sb[:, t:t + 1],
                        op0=mybir.AluOpType.mult, op1=mybir.AluOpType.add,
                    )
                else:
                    nc.vector.tensor_scalar_mul(dwt_col, wt_col, DEN_SCALE)
                nc.vector.tensor_scalar_add(xt[:, E:E + 1], dwt_col, 0.0)
                # q column: q * wt = q * denwt / DEN_SCALE
                nc.vector.tensor_scalar(
                    xt[:, 0:E], qw_sb[:, t, :], dwt_col,
                    1.0 / DEN_SCALE, op0=mybir.AluOpType.mult,
                    op1=mybir.AluOpType.mult,
                )

                oh2 = oh_groups[og]
                # leftovers of this tile into previous tile's bank (closes it)
                if t >= 1:
                    nc.tensor.matmul(
                        bank_ps[t - 1][0:SLOTS, :], oh2[:, ogi, 0:SLOTS],
                        xt[:], start=False, stop=True,
                    )
                    nc.scalar.copy(
                        bank_sb[0:SLOTS, t - 1, :], bank_ps[t - 1][0:SLOTS, :]
                    )
                ps = pseg.tile([128, 129], F32)
                bank_ps[t] = ps
                last = t == NT - 1
                nc.tensor.matmul(
                    ps[0:SLOTS, :], oh2[:, ogi, SLOTS:2 * SLOTS], xt[:],
                    start=True, stop=last,
                )
                if last:
                    nc.scalar.copy(bank_sb[0:SLOTS, t, :], ps[0:SLOTS, :])

            # ---- phase C: gather num/den per row, dot with q ----
            NOG = (NT + GO - 1) // GO

            def ensure_ohg(og):
                if og >= NOG or ("g", og) in oh_groups:
                    return
                t0 = og * GO
                ogn = min(GO, NT - t0)
                ogt = ohgpool.tile([128, GO, 256], FP8)
                nc.sync.dma_start(
                    ogt[0:SLOTS, 0:ogn, :], ohs_gat[:, t0:t0 + ogn, :]
                )
                oh_groups[("g", og)] = ogt

            def phase_c(t):
                og, ogi = divmod(t, GO)
                if ogi == 0:
                    ensure_ohg(og)
                g2 = oh_groups[("g", og)]
                gp = pgat.tile([128, 129], F32)
                if t >= 1:
                    nc.tensor.matmul(
                        gp[:], g2[0:SLOTS, ogi, 0:128],
                        bank_sb[0:SLOTS, t - 1, :],
                        start=True, stop=False,
                    )
                    nc.tensor.matmul(
                        gp[:], g2[0:SLOTS, ogi, 128:256],
                        bank_sb[0:SLOTS, t, :],
                        start=False, stop=True,
                    )
                else:
                    nc.tensor.matmul(
                        gp[:], g2[0:SLOTS, ogi, 128:256],
                        bank_sb[0:SLOTS, t, :],
                        start=True, stop=True,
                    )
                nc.scalar.copy(den_sb[:, t:t + 1], gp[:, E:E + 1])
                # rnum = sum_e num[user] * q, fused multiply+accumulate
                pq = zpool.tile([128, E], BF16, tag="pq")
                nc.vector.scalar_tensor_tensor(
                    pq[:], gp[:, 0:E], 0.0, qw_sb[:, t, :],
                    op0=mybir.AluOpType.add, op1=mybir.AluOpType.mult,
                    accum_out=rn_sb[:, t:t + 1],
                )
                if ogi == GO - 1 or t == NT - 1:
                    # finalize this group: r = rnum / den, stream it out
                    t0 = og * GO
                    gn = t - t0 + 1
                    rec = small.tile([128, GO], F32, tag="rec")
                    nc.vector.reciprocal(rec[:, 0:gn], den_sb[:, t0:t0 + gn])
                    nc.vector.tensor_mul(
                        r_sb[:, t0:t0 + gn], rn_sb[:, t0:t0 + gn],
                        rec[:, 0:gn],
                    )
                    # Pool-engine (SWDGE) queue: its dependency wait must not
                    # head-of-line block the R stream DMAs on the SP queue.
                    # The final group rides SP (cheaper, queue is empty then).
                    eng = nc.sync if t == NT - 1 else nc.gpsimd
                    eng.dma_start(
                        r_out[:, t0:t0 + gn], r_sb[:, t0:t0 + gn]
                    )

            # interleave: phase C lags phase A by LAG tiles so its DMAs and
            # matmuls overlap the R stream instead of queueing after it
            LAG = 3
            for it in range(NT + LAG):
                if it < NT:
                    phase_a(it)
                if it >= LAG:
                    phase_c(it - LAG)

    nc.compile()
    return nc


# ----------------------------------------------------------------------------
# entry point
# ----------------------------------------------------------------------------

def kernel(users, items, R_ui, mask, w, item_emb, alpha, beta, gamma,
           _return_extras=False, _trace=False):
    users = np.asarray(users, np.int64)
    items = np.asarray(items, np.int64)
    R_ui = np.asarray(R_ui, np.float32)
    mask_b = np.asarray(mask)
    mask_f = mask_b.astype(np.float32)
    w = np.asarray(w, np.float32)
    item_emb = np.asarray(item_emb, np.float32)
    al = float(np.asarray(alpha).reshape(-1)[0])
    be = float(np.asarray(beta).reshape(-1)[0])
    ga = float(np.asarray(gamma).reshape(-1)[0])

    import time as _time

    t0 = _time.perf_counter()
    in_maps, metas, NT = _preprocess(users, items, R_ui, mask_f, w, item_emb)
    t1 = _time.perf_counter()
    nc = build_program(NT, al, be, ga)
    t2 = _time.perf_counter()
    res = run_bass_kernel_spmd(
        nc, in_maps, core_ids=list(range(N_CORES)), trace=_trace
    )
    t3 = _time.perf_counter()
    print(
        f"[kernel] preprocess {t1-t0:.1f}s  build+schedule {t2-t1:.1f}s  "
        f"compile+run {t3-t2:.1f}s"
    )

    n = users.shape[0]
    r = np.empty(n, np.float32)
    for c in range(N_CORES):
        p, nc_rows = metas[c]
        shard = res.results[c]["r_out"].T.reshape(-1)[:nc_rows]
        r[p] = shard
    if _return_extras:
        return r, res
    return r

